# revision 40
# baseline (speedup 1.0000x reference)
"""ANI-style MoE routing kernel for 8 Trainium2 NeuronCores — v5.

Strategy (data-parallel + host routing):
  - Host: sort atoms by type, split each type's list evenly across 8 cores,
    build per-core per-expert contiguous batches padded to per-expert
    capacities (multiples of 1024).  Batches are feature-major bf16
    [384, sum(caps)].  Small overflows (<= SHED_MAX atoms) are computed
    exactly on the host in f64 instead of paying a whole device block.
  - Device (per core), per 1024-atom block, dim-major:
      mm1 (PE): z1 = W1^T x.  H1=192 -> m0 [128,1024] + m1 [64,1024]
        folded to [128,512] (two 512-atom column halves at partition
        bases 0/64, i.e. PE col-tiles (0,0)/(0,64)).
      celu1: t1 = exp(z1) [ACT, f32 — bf16 t1 fails the error gate: the
        t1-1 cancellation near 1.0 quantizes u1 to a few levels];
        u1 = min(t1-1,0) [DVE ts, 2x_2P]; g1 = (u1*1) max z1 [DVE stt]
        -> bf16, accum_out giving per-dim column sums S1 of g1.
        The m0 chain is split into column halves (P1_SPLIT) so g1k0-h0
        releases early for MM2 and z1m0 rotates sooner: -13us measured.
      mm2 (PE): z2 = W2^T g1.  H2=160 -> m0 [128,1024] + m1 packed as
        four 256-atom quarters [128,256] (PE col-tiles (.,32q)).
      layer-2 celu + reduction WITHOUT materializing celu(z2):
        sum_a celu(z2) = (sum z2 - sum min(z2,0)) + sum exp(min(z2,0)) - n
        sum z2 comes from S1 (host: W2^T S1, f64); the device computes
        m0 via m2 = min(z2,0) [DVE ts + accum_out -> Smin] then exp(m2)
        [ACT + accum_out -> Sexp]; the m1 quarters go pure-ACT (B-form):
        relu(-z2) [accum -> -Smin] then exp(-r) [accum -> Sexp].
  - Host: energy_e = w3^T (W2^T S1 - Smin + Sexp - n*1) (+ bias terms),
    summed over experts, + b3*counts + shed energies, in f64.

Zero-bias (always true for this problem's init) makes padding columns
self-cancelling: z=0 -> g1=0 contributes 0 to S1; min=0, exp(0)=1 cancels
against the -n term.  The general-bias path adds per-layer L1 bias matmuls,
applies b2 via the per-partition scalar operand of the m2 tensor_scalar,
and corrects pads on the host.

Measured on HW (slope of For_i repeat loop, 8 cores): 153.5-156.1us per body
(MM1 m1-first + MM2 kt1-first emission + P1/P2 half-block pointwise
splits) vs 184us baseline; rel err 2.2e-4 (gate 2e-2).  Engine balance per
1024-atom block (sim): PE 4.5us (streaming floor for serial 128x128
operation: 10240 cols @ 2.4GHz), DVE 4.2us, ACT 4.1us.  GPSIMD offload
was tried and abandoned: ~10us per instruction dispatch on this runtime.
"""

import os
import sys

import numpy as np

try:
    import concourse.bass as bass  # noqa: F401
except ImportError:  # pragma: no cover
    sys.path.insert(0, "/opt/trn_rl_repo")
    import concourse.bass as bass  # noqa: F401

import concourse.mybir as mybir
import concourse.tile as tile
from concourse import bacc
from concourse import bass_utils

IN_DIM = 384
H1 = 192
H2 = 160
E = 4
N_CORES = 8
N_ATOMS = 262144

BLOCK = 1024
HB = 512  # half block
QB = 256  # quarter block

F32 = mybir.dt.float32
BF16 = mybir.dt.bfloat16
AF = mybir.ActivationFunctionType
ALU = mybir.AluOpType

# engine assignment knobs (A/B-testable).  Defaults are the measured-best
# config: u1 on DVE (GPSIMD's per-instruction dispatch is ~10us on this
# runtime), P1 m0-chain split into column halves.
U1ENG = os.environ.get("U1ENG", "dve")  # dve | gpsimd
P1_SPLIT = os.environ.get("P1_SPLIT", "1") == "1"
P2_SPLIT = os.environ.get("P2_SPLIT", "1") == "1"
M1_SPLIT = os.environ.get("M1_SPLIT", "0") == "1"
# accum columns per block in outS: 6 base + P1 h1 spare + P2 h1 spare
COLS_PER_BLOCK = (6 + (1 if P1_SPLIT else 0) + (1 if P2_SPLIT else 0)
                  + (1 if M1_SPLIT else 0))


def _build_graph(with_bias: bool, caps, repeat: int = 1):
    nc = bacc.Bacc(
        "TRN2",
        target_bir_lowering=False,
        debug=False,
        enable_asserts=False,
        num_devices=N_CORES,
    )
    total_cap = sum(caps)
    nb = total_cap // BLOCK
    xT = nc.dram_tensor("xT", [IN_DIM, total_cap], BF16, kind="ExternalInput").ap()
    W1 = nc.dram_tensor("W1", [E, IN_DIM, H1], BF16, kind="ExternalInput").ap()
    W2 = nc.dram_tensor("W2", [E, H1, H2], BF16, kind="ExternalInput").ap()
    B1 = B2 = None
    if with_bias:
        B1 = nc.dram_tensor("B1", [E, H1], BF16, kind="ExternalInput").ap()
        B2 = nc.dram_tensor("B2", [E, H2], F32, kind="ExternalInput").ap()
    outS = nc.dram_tensor(
        "outS", [128, COLS_PER_BLOCK * nb], F32, kind="ExternalOutput"
    ).ap()

    with tile.TileContext(nc) as tc:
        _emit(tc, xT, W1, W2, B1, B2, outS, with_bias, caps, repeat)
    nc.compile()
    return nc


def _emit(tc, xT, W1, W2, B1, B2, outS, with_bias, caps, repeat=1):
    import contextlib

    nc = tc.nc
    xT3 = xT.rearrange("(kt kp) n -> kp kt n", kp=128)  # [128, 3, total]
    nb = sum(caps) // BLOCK

    with (
        tc.tile_pool(name="wpool", bufs=1) as wp,
        tc.tile_pool(name="xpool", bufs=int(os.environ.get("X_BUFS", "3"))) as xp,
        tc.tile_pool(name="t1pool", bufs=int(os.environ.get("SB_BUFS", "2"))) as t1p,
        tc.tile_pool(name="u1pool", bufs=int(os.environ.get("SB_BUFS", "2"))) as u1p,
        tc.tile_pool(name="g1pool", bufs=int(os.environ.get("SB_BUFS", "2"))) as g1p,
        tc.tile_pool(name="m2pool", bufs=int(os.environ.get("SB_BUFS", "2"))) as m2p,
        tc.tile_pool(name="e2pool", bufs=int(os.environ.get("SB_BUFS", "2"))) as e2p,
        tc.tile_pool(name="spool", bufs=1) as sp,
        tc.tile_pool(name="z1m0p", bufs=int(os.environ.get("Z1M0_BUFS", "2")),
                     space="PSUM") as z1m0p,
        tc.tile_pool(name="z1m1p", bufs=int(os.environ.get("Z1M1_BUFS", "1")),
                     space="PSUM") as z1m1p,
        tc.tile_pool(name="z2m0p", bufs=1, space="PSUM") as z2m0p,
        tc.tile_pool(name="z2m1p", bufs=1, space="PSUM") as z2m1p,
    ):
        # ---- persistent tiles (outside the timing repeat loop) ----
        w1s, w2s, b1s, b2m0s, b2m1s = [], [], [], [], []
        for e in range(E):
            w1 = wp.tile([128, 3, H1], BF16, tag=f"w1_{e}")
            nc.sync.dma_start(
                out=w1[:], in_=W1[e].rearrange("(kt kp) m -> kp kt m", kp=128)
            )
            w2 = wp.tile([128, 2, H2], BF16, tag=f"w2_{e}")
            nc.sync.dma_start(out=w2[:, 0, :], in_=W2[e][0:128, :])
            # kt1 weights duplicated at partition bases 0 and 64: matmul
            # requires lhsT.base_partition() == rhs.base_partition(), and
            # g1k1's two atom-halves live at partitions 0:64 / 64:128.
            nc.sync.dma_start(out=w2[0:64, 1, :], in_=W2[e][128:192, :])
            nc.sync.dma_start(out=w2[64:128, 1, :], in_=W2[e][128:192, :])
            w1s.append(w1)
            w2s.append(w2)
            if with_bias:
                b1 = wp.tile([1, H1], BF16, tag=f"b1_{e}")
                nc.sync.dma_start(out=b1[:], in_=B1[e : e + 1, :])
                b1s.append(b1)
                # per-partition bias columns for the L2 tensor_scalar:
                # m0: dims 0:128; m1 quarters: dim 128+j at partition 32q+j
                b2m0 = wp.tile([128, 1], F32, tag=f"b2m0_{e}")
                nc.sync.dma_start(
                    out=b2m0[:], in_=B2[e, 0:128].rearrange("(p one) -> p one", one=1)
                )
                b2m1 = wp.tile([128, 1], F32, tag=f"b2m1_{e}")
                for q in range(4):
                    nc.sync.dma_start(
                        out=b2m1[32 * q : 32 * q + 32, :],
                        in_=B2[e, 128:160].rearrange("(p one) -> p one", one=1),
                    )
                b2m0s.append(b2m0)
                b2m1s.append(b2m1)
        ones = None
        if with_bias:
            ones = sp.tile([1, HB], BF16, tag="ones")
            nc.vector.memset(ones[:], 1.0)
        SD = sp.tile([128, COLS_PER_BLOCK * nb], F32, tag="SD")
        nc.vector.memset(SD[:], 0.0)

        loop_cm = tc.For_i(0, repeat, 1) if repeat > 1 else contextlib.nullcontext()
        with loop_cm:
            _emit_body(
                tc, xT3, w1s, w2s, b1s, b2m0s, b2m1s, ones, SD, with_bias, caps,
                xp, t1p, u1p, g1p, m2p, e2p, z1m0p, z1m1p, z2m0p, z2m1p,
            )
        nc.sync.dma_start(out=outS, in_=SD[:])


def _emit_body(
    tc, xT3, w1s, w2s, b1s, b2m0s, b2m1s, ones, SD, with_bias, caps,
    xp, t1p, u1p, g1p, m2p, e2p, z1m0p, z1m1p, z2m0p, z2m1p,
):
    nc = tc.nc
    CB = COLS_PER_BLOCK
    ONLY = os.environ.get("ONLY", "full")
    nb = sum(caps) // BLOCK
    block_expert = []
    for e in range(E):
        block_expert += [e] * (caps[e] // BLOCK)

    u1eng = nc.gpsimd if U1ENG == "gpsimd" else nc.vector

    PREFETCH = int(os.environ.get("PREFETCH", "2"))
    S = {}  # per-block pipeline state

    def dma_x(i):
        xa = xp.tile([128, 3, BLOCK], BF16, tag="xa")
        nc.sync.dma_start(
            out=xa[:], in_=xT3[:, :, i * BLOCK : (i + 1) * BLOCK]
        )
        return xa

    for i in range(min(PREFETCH, nb)):
        S[i] = {"xa": dma_x(i)}

    STAGE_ORDER = os.environ.get("STAGE_ORDER", "p1_p2")

    def do_p1_m1(j):
        # t1/u1 must stay f32: u1 = t1-1 with t1 near 1.0 — bf16's
        # 2^-8 grid at 1.0 quantizes u1 (|u1| <= ~0.03 here) to a few
        # levels and the S1 sums blow past the error gate.
        st = S[j]
        z1m1 = st["z1m1"]
        t1 = t1p.tile([128, 1536], F32, tag="t1")
        u1 = u1p.tile([128, 1536], F32, tag="u1")
        g1k0 = g1p.tile([128, BLOCK], BF16, tag="g1k0")
        g1k1 = g1p.tile([128, HB], BF16, tag="g1k1")
        st["t1"], st["u1"] = t1, u1
        st["g1k0"], st["g1k1"] = g1k0, g1k1
        # m1 first: unblocks mm1m1(i) (single-buffered Z1M1) earliest
        if M1_SPLIT:
            # column halves: STT-m1-h0 completes while exp-h1 runs, so
            # z1m1 releases ~300ns earlier for MM1m1(i+1)
            for h in range(2):
                lo, hi = 1024 + QB * h, 1024 + QB * (h + 1)
                nc.scalar.activation(t1[:, lo:hi], z1m1[:, QB * h : QB * (h + 1)], AF.Exp)
                u1eng.tensor_scalar(
                    out=u1[:, lo:hi], in0=t1[:, lo:hi],
                    scalar1=-1.0, scalar2=0.0, op0=ALU.add, op1=ALU.min,
                )
                nc.vector.scalar_tensor_tensor(
                    out=g1k1[:, QB * h : QB * (h + 1)], in0=u1[:, lo:hi],
                    scalar=1.0, in1=z1m1[:, QB * h : QB * (h + 1)],
                    op0=ALU.mult, op1=ALU.max,
                    accum_out=SD[:, CB * j + 1 : CB * j + 2]
                    if h == 0 else SD[:, CB * j + 8 : CB * j + 9],
                )
        else:
            nc.scalar.activation(t1[:, 1024:1536], z1m1[:], AF.Exp)
            u1eng.tensor_scalar(
                out=u1[:, 1024:1536], in0=t1[:, 1024:1536],
                scalar1=-1.0, scalar2=0.0, op0=ALU.add, op1=ALU.min,
            )
            nc.vector.scalar_tensor_tensor(
                out=g1k1[:], in0=u1[:, 1024:1536], scalar=1.0, in1=z1m1[:],
                op0=ALU.mult, op1=ALU.max,
                accum_out=SD[:, CB * j + 1 : CB * j + 2],
            )

    def do_p1_m0(j):
        st = S[j]
        z1m0 = st["z1m0"]
        t1, u1, g1k0 = st["t1"], st["u1"], st["g1k0"]
        nc.scalar.activation(t1[:, 0:1024], z1m0[:], AF.Exp)
        if P1_SPLIT:
            # halve the m0 chain: g1k0-h0 releases early so MM2(i)'s
            # kt0-m0-h0 can start while h1 is still in pointwise.
            # accum columns per half are summed on the host (same dims).
            for h in range(2):
                u1eng.tensor_scalar(
                    out=u1[:, HB * h : HB * (h + 1)],
                    in0=t1[:, HB * h : HB * (h + 1)],
                    scalar1=-1.0, scalar2=0.0, op0=ALU.add, op1=ALU.min,
                )
                nc.vector.scalar_tensor_tensor(
                    out=g1k0[:, HB * h : HB * (h + 1)],
                    in0=u1[:, HB * h : HB * (h + 1)], scalar=1.0,
                    in1=z1m0[:, HB * h : HB * (h + 1)],
                    op0=ALU.mult, op1=ALU.max,
                    accum_out=SD[:, CB * j + 0 : CB * j + 1]
                    if h == 0 else SD[:, CB * j + 6 : CB * j + 7],
                )
        else:
            u1eng.tensor_scalar(
                out=u1[:, 0:1024], in0=t1[:, 0:1024],
                scalar1=-1.0, scalar2=0.0, op0=ALU.add, op1=ALU.min,
            )
            nc.vector.scalar_tensor_tensor(
                out=g1k0[:], in0=u1[:, 0:1024], scalar=1.0, in1=z1m0[:],
                op0=ALU.mult, op1=ALU.max,
                accum_out=SD[:, CB * j + 0 : CB * j + 1],
            )

    DMA_FIRST = os.environ.get("DMA_FIRST", "0") == "1"
    for i in range(nb + 2):
        j = i - 1
        k = i - 2
        if DMA_FIRST and i + PREFETCH < nb:
            S[i + PREFETCH] = {"xa": dma_x(i + PREFETCH)}
        p1_ok = 0 <= j < nb and ONLY in ("p1", "mm2", "full")
        # ---- stage P1(i-1) m1 chain (latency-critical) ----
        if p1_ok:
            do_p1_m1(j)
        if STAGE_ORDER == "p1_p2" and p1_ok:
            do_p1_m0(j)

        # ---- stage P2(i-2): layer-2 min/exp + fused column sums ----
        if 0 <= k < nb and ONLY == "full":
            st = S.pop(k)
            e = block_expert[k]
            z2m0, z2m1 = st["z2m0"], st["z2m1"]
            m2 = m2p.tile([128, 1280], F32, tag="m2")
            e2 = e2p.tile([128, 1280], F32, tag="e2")
            if with_bias:
                # m1 A-form: m2 = min(z2 + b2, 0); b2 per-partition scalar AP
                nc.vector.tensor_scalar(
                    out=m2[:, 1024:1280], in0=z2m1[:],
                    scalar1=b2m1s[e][:], scalar2=0.0, op0=ALU.add, op1=ALU.min,
                    accum_out=SD[:, CB * k + 3 : CB * k + 4],
                )
                nc.scalar.activation(
                    e2[:, 1024:1280], m2[:, 1024:1280], AF.Exp,
                    accum_out=SD[:, CB * k + 5 : CB * k + 6],
                )
                nc.vector.tensor_scalar(
                    out=m2[:, 0:1024], in0=z2m0[:],
                    scalar1=b2m0s[e][:], scalar2=0.0, op0=ALU.add, op1=ALU.min,
                    accum_out=SD[:, CB * k + 2 : CB * k + 3],
                )
            else:
                # m1 B-form, pure ACT: r2 = relu(-z2) (accum = -Smin_m1),
                # then exp(-r2) = exp(min(z2,0)) (accum = Sexp_m1).
                nc.scalar.activation(
                    m2[:, 1024:1280], z2m1[:], AF.Relu, scale=-1.0,
                    accum_out=SD[:, CB * k + 3 : CB * k + 4],
                )
                nc.scalar.activation(
                    e2[:, 1024:1280], m2[:, 1024:1280], AF.Exp, scale=-1.0,
                    accum_out=SD[:, CB * k + 5 : CB * k + 6],
                )
                if P2_SPLIT and P1_SPLIT:
                    # halves: TS-h0 starts after kt1-h0 stop; both accum
                    # columns carry the same dims (summed with col 2/6 pairs
                    # on the host via the h1 spare column)
                    nc.vector.tensor_scalar(
                        out=m2[:, 0:512], in0=z2m0[:, 0:512],
                        scalar1=0.0, scalar2=0.0, op0=ALU.min, op1=ALU.add,
                        accum_out=SD[:, CB * k + 2 : CB * k + 3],
                    )
                    nc.vector.tensor_scalar(
                        out=m2[:, 512:1024], in0=z2m0[:, 512:1024],
                        scalar1=0.0, scalar2=0.0, op0=ALU.min, op1=ALU.add,
                        accum_out=SD[:, CB * k + 7 : CB * k + 8],
                    )
                else:
                    nc.vector.tensor_scalar(
                        out=m2[:, 0:1024], in0=z2m0[:],
                        scalar1=0.0, scalar2=0.0, op0=ALU.min, op1=ALU.add,
                        accum_out=SD[:, CB * k + 2 : CB * k + 3],
                    )
            nc.scalar.activation(
                e2[:, 0:1024], m2[:, 0:1024], AF.Exp,
                accum_out=SD[:, CB * k + 4 : CB * k + 5],
            )

        # ---- stage P1(i-1) m0 chain, after P2 when reordered ----
        if STAGE_ORDER != "p1_p2" and p1_ok:
            do_p1_m0(j)

        # ---- stage MM1(i) ----
        if i < nb and ONLY != "dma":
            st = S[i]
            e = block_expert[i]
            w1 = w1s[e]
            xa = st["xa"]
            z1m0 = z1m0p.tile([128, BLOCK], F32, tag="z1m0")
            z1m1 = z1m1p.tile([128, HB], F32, tag="z1m1")
            last = 2 if not with_bias else -1
            MM1_ORDER = os.environ.get("MM1_ORDER", "m1_first")
            if MM1_ORDER == "m1_first":
                for kt in range(3):
                    for h in range(2):
                        nc.tensor.matmul(
                            z1m1[64 * h : 64 * (h + 1), :],
                            lhsT=w1[:, kt, 128:192],
                            rhs=xa[:, kt, HB * h : HB * (h + 1)],
                            start=(kt == 0), stop=(kt == last),
                        )
                for kt in range(3):
                    for h in range(2):
                        nc.tensor.matmul(
                            z1m0[:, HB * h : HB * (h + 1)],
                            lhsT=w1[:, kt, 0:128],
                            rhs=xa[:, kt, HB * h : HB * (h + 1)],
                            start=(kt == 0), stop=(kt == last),
                        )
            elif MM1_ORDER == "interleave":
                for kt in range(3):
                    for h in range(2):
                        nc.tensor.matmul(
                            z1m0[:, HB * h : HB * (h + 1)],
                            lhsT=w1[:, kt, 0:128],
                            rhs=xa[:, kt, HB * h : HB * (h + 1)],
                            start=(kt == 0), stop=(kt == last),
                        )
                    for h in range(2):
                        nc.tensor.matmul(
                            z1m1[64 * h : 64 * (h + 1), :],
                            lhsT=w1[:, kt, 128:192],
                            rhs=xa[:, kt, HB * h : HB * (h + 1)],
                            start=(kt == 0), stop=(kt == last),
                        )
            else:  # m0 first, m1 grouped at the end
                for kt in range(3):
                    for h in range(2):
                        nc.tensor.matmul(
                            z1m0[:, HB * h : HB * (h + 1)],
                            lhsT=w1[:, kt, 0:128],
                            rhs=xa[:, kt, HB * h : HB * (h + 1)],
                            start=(kt == 0), stop=(kt == last),
                        )
                for kt in range(3):
                    for h in range(2):
                        nc.tensor.matmul(
                            z1m1[64 * h : 64 * (h + 1), :],
                            lhsT=w1[:, kt, 128:192],
                            rhs=xa[:, kt, HB * h : HB * (h + 1)],
                            start=(kt == 0), stop=(kt == last),
                        )
            if with_bias:
                b1 = b1s[e]
                for h in range(2):
                    nc.tensor.matmul(
                        z1m0[:, HB * h : HB * (h + 1)], lhsT=b1[:, 0:128],
                        rhs=ones[:], start=False, stop=True,
                    )
                for h in range(2):
                    nc.tensor.matmul(
                        z1m1[64 * h : 64 * (h + 1), :], lhsT=b1[:, 128:192],
                        rhs=ones[:], start=False, stop=True,
                    )
            st["z1m0"], st["z1m1"] = z1m0, z1m1

        # ---- stage MM2(i-1) ----
        if 0 <= j < nb and ONLY in ("mm2", "full"):
            st = S[j]
            e = block_expert[j]
            w2 = w2s[e]
            g1k0, g1k1 = st["g1k0"], st["g1k1"]
            z2m0 = z2m0p.tile([128, BLOCK], F32, tag="z2m0")
            # m1 packed as four 256-atom quarters: dim 128+d at
            # partition 32q+d, atom = 256q + col
            z2m1 = z2m1p.tile([128, QB], F32, tag="z2m1")
            if os.environ.get("MM2_ORDER", "kt1_first") == "kt1_first":
                # kt1 depends only on g1k1 (its STT completes before
                # g1k0's in P1), so its matmuls are ready first.
                for h in range(2):
                    nc.tensor.matmul(
                        z2m0[:, HB * h : HB * (h + 1)],
                        lhsT=w2[64 * h : 64 * h + 64, 1, 0:128],
                        rhs=g1k1[64 * h : 64 * (h + 1), :],
                        start=True, stop=False,
                    )
                for q in range(4):
                    h, r = q >> 1, q & 1
                    nc.tensor.matmul(
                        z2m1[32 * q : 32 * q + 32, :],
                        lhsT=w2[64 * h : 64 * h + 64, 1, 128:160],
                        rhs=g1k1[64 * h : 64 * (h + 1), QB * r : QB * (r + 1)],
                        start=True, stop=False,
                        tile_position=(64 * h, 32 * q),
                    )
                for h in range(2):
                    nc.tensor.matmul(
                        z2m0[:, HB * h : HB * (h + 1)], lhsT=w2[:, 0, 0:128],
                        rhs=g1k0[:, HB * h : HB * (h + 1)],
                        start=False, stop=True,
                    )
                for q in range(4):
                    nc.tensor.matmul(
                        z2m1[32 * q : 32 * q + 32, :],
                        lhsT=w2[:, 0, 128:160],
                        rhs=g1k0[:, QB * q : QB * (q + 1)],
                        start=False, stop=True,
                        tile_position=(0, 32 * q),
                    )
            else:
                # kt0 m0 (LDW w2[:,0,0:128])
                for h in range(2):
                    nc.tensor.matmul(
                        z2m0[:, HB * h : HB * (h + 1)], lhsT=w2[:, 0, 0:128],
                        rhs=g1k0[:, HB * h : HB * (h + 1)],
                        start=True, stop=False,
                    )
                # kt0 m1: quarter q at PE col-tile (0, 32q)
                for q in range(4):
                    nc.tensor.matmul(
                        z2m1[32 * q : 32 * q + 32, :],
                        lhsT=w2[:, 0, 128:160],
                        rhs=g1k0[:, QB * q : QB * (q + 1)],
                        start=True, stop=False,
                        tile_position=(0, 32 * q),
                    )
                # kt1 m0; g1k1 parts: h half of atoms, lhsT at matching base
                if os.environ.get("KT1Q", "0") == "1":
                    for h in range(2):
                        for c in range(2):
                            nc.tensor.matmul(
                                z2m0[:, HB * h + QB * c : HB * h + QB * (c + 1)],
                                lhsT=w2[64 * h : 64 * h + 64, 1, 0:128],
                                rhs=g1k1[64 * h : 64 * (h + 1), QB * c : QB * (c + 1)],
                                start=False, stop=True,
                            )
                else:
                    for h in range(2):
                        nc.tensor.matmul(
                            z2m0[:, HB * h : HB * (h + 1)],
                            lhsT=w2[64 * h : 64 * h + 64, 1, 0:128],
                            rhs=g1k1[64 * h : 64 * (h + 1), :],
                            start=False, stop=True,
                        )
                # kt1 m1: quarter q = (h, r): rhs g1k1[64h:64h+64, 256r:..],
                # PE tile (64h, 32q) — row+col compose
                for q in range(4):
                    h, r = q >> 1, q & 1
                    nc.tensor.matmul(
                        z2m1[32 * q : 32 * q + 32, :],
                        lhsT=w2[64 * h : 64 * h + 64, 1, 128:160],
                        rhs=g1k1[64 * h : 64 * (h + 1), QB * r : QB * (r + 1)],
                        start=False, stop=True,
                        tile_position=(64 * h, 32 * q),
                    )
            st["z2m0"], st["z2m1"] = z2m0, z2m1

        # ---- prefetch ----
        if not DMA_FIRST:
            nxt = i + PREFETCH
            if nxt < nb:
                S[nxt] = {"xa": dma_x(nxt)}


_GRAPH_CACHE = {}


def _get_graph(with_bias: bool, caps):
    key = (with_bias, tuple(caps))
    if key not in _GRAPH_CACHE:
        _GRAPH_CACHE[key] = _build_graph(with_bias, caps)
    return _GRAPH_CACHE[key]


def _celu64(v):
    return np.where(v > 0, v, np.expm1(np.minimum(v, 0.0)))


def prepare_in_maps(aev_inputs, atom_types, W1, b1, W2, b2, W3, b3):
    """Host routing: build per-core input maps + metadata for decode."""
    import ml_dtypes

    ndt = ml_dtypes.bfloat16
    aev = np.asarray(aev_inputs, dtype=np.float32)
    types = np.asarray(atom_types).astype(np.int64)
    W1f = np.asarray(W1, dtype=np.float32)
    b1 = np.asarray(b1, dtype=np.float32)
    W2f = np.asarray(W2, dtype=np.float32)
    b2 = np.asarray(b2, dtype=np.float32)
    W3f = np.asarray(W3, dtype=np.float32)
    b3 = np.asarray(b3, dtype=np.float32)
    W1b = np.ascontiguousarray(W1f.astype(ndt))
    W2b = np.ascontiguousarray(W2f.astype(ndt))

    with_bias = bool(np.any(b1) or np.any(b2))

    order = np.argsort(types, kind="stable")
    sorted_types = types[order]
    bounds = np.searchsorted(sorted_types, np.arange(E + 1))
    type_lists = [order[bounds[e] : bounds[e + 1]] for e in range(E)]

    SHED_MAX = 192
    slices = [[None] * E for _ in range(N_CORES)]
    n_real = np.zeros((N_CORES, E), dtype=np.int64)
    shed = []
    caps = []
    for e in range(E):
        lst = type_lists[e]
        counts = [
            ((len(lst) * (c + 1)) // N_CORES) - ((len(lst) * c) // N_CORES)
            for c in range(N_CORES)
        ]
        mx = max(counts)
        rem = mx % BLOCK
        if 0 < rem <= SHED_MAX:
            cap_e = (mx // BLOCK) * BLOCK
        else:
            cap_e = -(-mx // BLOCK) * BLOCK
        caps.append(cap_e)
        for c in range(N_CORES):
            lo = (len(lst) * c) // N_CORES
            hi = (len(lst) * (c + 1)) // N_CORES
            take = min(hi - lo, cap_e)
            slices[c][e] = lst[lo : lo + take]
            shed.append(lst[lo + take : hi])
            n_real[c, e] = take
    shed = np.concatenate(shed) if shed else np.zeros(0, dtype=np.int64)
    caps = tuple(caps)
    offs = np.cumsum([0] + list(caps))

    shed_energy = 0.0
    if len(shed):
        xs = aev[shed].astype(np.float64)
        ts_ = types[shed]
        for e in range(E):
            m = ts_ == e
            if not m.any():
                continue
            h = _celu64(xs[m] @ W1f[e].astype(np.float64) + b1[e].astype(np.float64))
            h = _celu64(h @ W2f[e].astype(np.float64) + b2[e].astype(np.float64))
            y = h @ W3f[e].astype(np.float64)[:, 0] + float(b3[e][0])
            shed_energy += float(y.sum())

    in_maps = []
    for c in range(N_CORES):
        xcT = np.zeros((IN_DIM, int(offs[-1])), dtype=ndt)
        for e in range(E):
            idx = slices[c][e]
            xcT[:, int(offs[e]) : int(offs[e]) + len(idx)] = aev[idx].T.astype(ndt)
        m = {"xT": xcT, "W1": W1b, "W2": W2b}
        if with_bias:
            m["B1"] = np.ascontiguousarray(b1.astype(ndt))
            m["B2"] = np.ascontiguousarray(b2.astype(np.float32))
        in_maps.append(m)
    return in_maps, n_real, with_bias, (b1, W2f, b2, W3f, b3, shed_energy), caps


def postprocess(results, n_real, wdata, caps, with_bias=False):
    """Decode per-block accum columns -> per-expert energies (f64).

    Per block k, SD columns 6k..6k+5 hold:
      0: S1_m0   [128]  sum over atoms of g1 (dims 0:128)
      1: S1_m1   [128]  folded: dim 128+j = col[j] + col[64+j]
      2: Smin_m0 [128]  sum of min(z2+b2, 0), dims 0:128
      3: Smin_m1 [128]  quarters: dim 128+j = sum_q col[32q+j]
         (zero-bias B-form: column holds sum relu(-z2) = -Smin_m1)
      4: Sexp_m0 [128]  sum of exp(min(z2+b2, 0))
      5: Sexp_m1 [128]  quarters, as 3
    """
    b1, W2f, b2, W3f, b3, shed_energy = wdata
    CB = COLS_PER_BLOCK
    nb = sum(caps) // BLOCK
    block_expert = []
    for e in range(E):
        block_expert += [e] * (caps[e] // BLOCK)

    S1 = np.zeros((E, H1), dtype=np.float64)
    Smin = np.zeros((E, H2), dtype=np.float64)
    Sexp = np.zeros((E, H2), dtype=np.float64)
    for c in range(N_CORES):
        D = np.asarray(results[c]["outS"], dtype=np.float64)
        for k in range(nb):
            e = block_expert[k]
            S1[e, 0:128] += D[:, CB * k]
            if P1_SPLIT:
                S1[e, 0:128] += D[:, CB * k + 6]
            if P2_SPLIT:
                Smin[e, 0:128] += D[:, CB * k + 7]
            S1[e, 128:192] += D[0:64, CB * k + 1] + D[64:128, CB * k + 1]
            if M1_SPLIT:
                S1[e, 128:192] += D[0:64, CB * k + 8] + D[64:128, CB * k + 8]
            Smin[e, 0:128] += D[:, CB * k + 2]
            m1col = D[:, CB * k + 3].reshape(4, 32).sum(axis=0)
            Smin[e, 128:160] += m1col if with_bias else -m1col
            Sexp[e, 0:128] += D[:, CB * k + 4]
            Sexp[e, 128:160] += D[:, CB * k + 5].reshape(4, 32).sum(axis=0)

    total = shed_energy
    counts_e = n_real.sum(axis=0)
    for e in range(E):
        ncols = float(N_CORES * caps[e])  # real + pad columns on device
        pads = ncols - float(counts_e[e])
        w3 = W3f[e].astype(np.float64)[:, 0]
        b2e = b2[e].astype(np.float64)
        # sum z2 over all device columns: W2^T S1 + ncols*b2
        sz2 = W2f[e].astype(np.float64).T @ S1[e] + ncols * b2e
        S = (sz2 - Smin[e]) + Sexp[e] - ncols  # sum of celu(z2+b2) per dim
        total += float(w3 @ S)
        total += float(counts_e[e]) * float(b3[e][0])
        if pads:
            # device pads contribute celu(z2_0) per dim; subtract (f64 model)
            h1 = _celu64(b1[e].astype(np.float64))
            z2_0 = h1 @ W2f[e].astype(np.float64) + b2e
            total -= pads * float(w3 @ _celu64(z2_0))
    return np.asarray(total, dtype=np.float32)


def kernel(aev_inputs, atom_types, W1, b1, W2, b2, W3, b3):
    in_maps, n_real, with_bias, wdata, caps = prepare_in_maps(
        aev_inputs, atom_types, W1, b1, W2, b2, W3, b3
    )
    nc = _get_graph(with_bias, caps)
    results = bass_utils.run_bass_kernel_spmd(
        nc, in_maps, core_ids=list(range(N_CORES))
    ).results
    return postprocess(results, n_real, wdata, caps, with_bias)


# revision 42
# speedup vs baseline: 1.0051x; 1.0051x over previous
"""ANI-style MoE routing kernel for 8 Trainium2 NeuronCores — v5.

Strategy (data-parallel + host routing):
  - Host: sort atoms by type, split each type's list evenly across 8 cores,
    build per-core per-expert contiguous batches padded to per-expert
    capacities (multiples of 1024).  Batches are feature-major bf16
    [384, sum(caps)].  Small overflows (<= SHED_MAX atoms) are computed
    exactly on the host in f64 instead of paying a whole device block.
  - Device (per core), per 1024-atom block, dim-major:
      mm1 (PE): z1 = W1^T x.  H1=192 -> m0 [128,1024] + m1 [64,1024]
        folded to [128,512] (two 512-atom column halves at partition
        bases 0/64, i.e. PE col-tiles (0,0)/(0,64)).
      celu1: t1 = exp(z1) [ACT, f32 — bf16 t1 fails the error gate: the
        t1-1 cancellation near 1.0 quantizes u1 to a few levels];
        u1 = min(t1-1,0) [DVE ts, 2x_2P]; g1 = (u1*1) max z1 [DVE stt]
        -> bf16, accum_out giving per-dim column sums S1 of g1.
        The m0 chain is split into column halves (P1_SPLIT) so g1k0-h0
        releases early for MM2 and z1m0 rotates sooner: -13us measured.
      mm2 (PE): z2 = W2^T g1.  H2=160 -> m0 [128,1024] + m1 packed as
        four 256-atom quarters [128,256] (PE col-tiles (.,32q)).
      layer-2 celu + reduction WITHOUT materializing celu(z2):
        sum_a celu(z2) = (sum z2 - sum min(z2,0)) + sum exp(min(z2,0)) - n
        sum z2 comes from S1 (host: W2^T S1, f64); the device computes
        m0 via m2 = min(z2,0) [DVE ts + accum_out -> Smin] then exp(m2)
        [ACT + accum_out -> Sexp]; the m1 quarters go pure-ACT (B-form):
        relu(-z2) [accum -> -Smin] then exp(-r) [accum -> Sexp].
  - Host: energy_e = w3^T (W2^T S1 - Smin + Sexp - n*1) (+ bias terms),
    summed over experts, + b3*counts + shed energies, in f64.

Zero-bias (always true for this problem's init) makes padding columns
self-cancelling: z=0 -> g1=0 contributes 0 to S1; min=0, exp(0)=1 cancels
against the -n term.  The general-bias path adds per-layer L1 bias matmuls,
applies b2 via the per-partition scalar operand of the m2 tensor_scalar,
and corrects pads on the host.

Measured on HW (slope of For_i repeat loop, 8 cores): 153.5-156.1us per body
(MM1 m1-first + MM2 kt1-first emission + P1/P2 half-block pointwise
splits) vs 184us baseline; rel err 2.2e-4 (gate 2e-2).  Engine balance per
1024-atom block (sim): PE 4.5us (streaming floor for serial 128x128
operation: 10240 cols @ 2.4GHz), DVE 4.2us, ACT 4.1us.  GPSIMD offload
was tried and abandoned: ~10us per instruction dispatch on this runtime.
"""

import os
import sys

import numpy as np

try:
    import concourse.bass as bass  # noqa: F401
except ImportError:  # pragma: no cover
    sys.path.insert(0, "/opt/trn_rl_repo")
    import concourse.bass as bass  # noqa: F401

import concourse.mybir as mybir
import concourse.tile as tile
from concourse import bacc
from concourse import bass_utils

IN_DIM = 384
H1 = 192
H2 = 160
E = 4
N_CORES = 8
N_ATOMS = 262144

BLOCK = 1024
HB = 512  # half block
QB = 256  # quarter block

F32 = mybir.dt.float32
BF16 = mybir.dt.bfloat16
AF = mybir.ActivationFunctionType
ALU = mybir.AluOpType

# engine assignment knobs (A/B-testable).  Defaults are the measured-best
# config: u1 on DVE (GPSIMD's per-instruction dispatch is ~10us on this
# runtime), P1 m0-chain split into column halves.
U1ENG = os.environ.get("U1ENG", "dve")  # dve | gpsimd
P1_SPLIT = os.environ.get("P1_SPLIT", "1") == "1"
P2_SPLIT = os.environ.get("P2_SPLIT", "1") == "1"
M1_SPLIT = os.environ.get("M1_SPLIT", "0") == "1"
# accum columns per block in outS: 6 base + P1 h1 spare + P2 h1 spare
COLS_PER_BLOCK = (6 + (1 if P1_SPLIT else 0) + (1 if P2_SPLIT else 0)
                  + (1 if M1_SPLIT else 0))


def _build_graph(with_bias: bool, caps, repeat: int = 1):
    nc = bacc.Bacc(
        "TRN2",
        target_bir_lowering=False,
        debug=False,
        enable_asserts=False,
        num_devices=N_CORES,
    )
    total_cap = sum(caps)
    nb = total_cap // BLOCK
    xT = nc.dram_tensor("xT", [IN_DIM, total_cap], BF16, kind="ExternalInput").ap()
    W1 = nc.dram_tensor("W1", [E, IN_DIM, H1], BF16, kind="ExternalInput").ap()
    W2 = nc.dram_tensor("W2", [E, H1, H2], BF16, kind="ExternalInput").ap()
    B1 = B2 = None
    if with_bias:
        B1 = nc.dram_tensor("B1", [E, H1], BF16, kind="ExternalInput").ap()
        B2 = nc.dram_tensor("B2", [E, H2], F32, kind="ExternalInput").ap()
    outS = nc.dram_tensor(
        "outS", [128, COLS_PER_BLOCK * nb], F32, kind="ExternalOutput"
    ).ap()

    with tile.TileContext(nc) as tc:
        _emit(tc, xT, W1, W2, B1, B2, outS, with_bias, caps, repeat)
    nc.compile()
    return nc


def _emit(tc, xT, W1, W2, B1, B2, outS, with_bias, caps, repeat=1):
    import contextlib

    nc = tc.nc
    xT3 = xT.rearrange("(kt kp) n -> kp kt n", kp=128)  # [128, 3, total]
    nb = sum(caps) // BLOCK

    with (
        tc.tile_pool(name="wpool", bufs=1) as wp,
        tc.tile_pool(name="xpool", bufs=int(os.environ.get("X_BUFS", "3"))) as xp,
        tc.tile_pool(name="t1pool", bufs=int(os.environ.get("SB_BUFS", "2"))) as t1p,
        tc.tile_pool(name="u1pool", bufs=int(os.environ.get("SB_BUFS", "2"))) as u1p,
        tc.tile_pool(name="g1pool", bufs=int(os.environ.get("SB_BUFS", "2"))) as g1p,
        tc.tile_pool(name="m2pool", bufs=int(os.environ.get("SB_BUFS", "2"))) as m2p,
        tc.tile_pool(name="e2pool", bufs=int(os.environ.get("SB_BUFS", "2"))) as e2p,
        tc.tile_pool(name="spool", bufs=1) as sp,
        tc.tile_pool(name="z1m0p", bufs=int(os.environ.get("Z1M0_BUFS", "2")),
                     space="PSUM") as z1m0p,
        tc.tile_pool(name="z1m1p", bufs=int(os.environ.get("Z1M1_BUFS", "1")),
                     space="PSUM") as z1m1p,
        tc.tile_pool(name="z2m0p", bufs=1, space="PSUM") as z2m0p,
        tc.tile_pool(name="z2m1p", bufs=1, space="PSUM") as z2m1p,
    ):
        # ---- persistent tiles (outside the timing repeat loop) ----
        w1s, w2s, b1s, b2m0s, b2m1s = [], [], [], [], []
        for e in range(E):
            w1 = wp.tile([128, 3, H1], BF16, tag=f"w1_{e}")
            nc.sync.dma_start(
                out=w1[:], in_=W1[e].rearrange("(kt kp) m -> kp kt m", kp=128)
            )
            w2 = wp.tile([128, 2, H2], BF16, tag=f"w2_{e}")
            nc.sync.dma_start(out=w2[:, 0, :], in_=W2[e][0:128, :])
            # kt1 weights duplicated at partition bases 0 and 64: matmul
            # requires lhsT.base_partition() == rhs.base_partition(), and
            # g1k1's two atom-halves live at partitions 0:64 / 64:128.
            nc.sync.dma_start(out=w2[0:64, 1, :], in_=W2[e][128:192, :])
            nc.sync.dma_start(out=w2[64:128, 1, :], in_=W2[e][128:192, :])
            w1s.append(w1)
            w2s.append(w2)
            if with_bias:
                b1 = wp.tile([1, H1], BF16, tag=f"b1_{e}")
                nc.sync.dma_start(out=b1[:], in_=B1[e : e + 1, :])
                b1s.append(b1)
                # per-partition bias columns for the L2 tensor_scalar:
                # m0: dims 0:128; m1 quarters: dim 128+j at partition 32q+j
                b2m0 = wp.tile([128, 1], F32, tag=f"b2m0_{e}")
                nc.sync.dma_start(
                    out=b2m0[:], in_=B2[e, 0:128].rearrange("(p one) -> p one", one=1)
                )
                b2m1 = wp.tile([128, 1], F32, tag=f"b2m1_{e}")
                for q in range(4):
                    nc.sync.dma_start(
                        out=b2m1[32 * q : 32 * q + 32, :],
                        in_=B2[e, 128:160].rearrange("(p one) -> p one", one=1),
                    )
                b2m0s.append(b2m0)
                b2m1s.append(b2m1)
        ones = None
        if with_bias:
            ones = sp.tile([1, HB], BF16, tag="ones")
            nc.vector.memset(ones[:], 1.0)
        SD = sp.tile([128, COLS_PER_BLOCK * nb], F32, tag="SD")
        nc.vector.memset(SD[:], 0.0)

        # WRAP: the initial x prefetches live OUTSIDE the repeat loop; the
        # body's tail wraparound DMAs (block (i+PREFETCH) mod nb) then feed
        # the next iteration's first consumers, eliminating per-iteration
        # pipeline refill.  Requires X_BUFS | nb so slot phase repeats.
        WRAP = os.environ.get("WRAP", "0") == "1"
        PREFETCH = int(os.environ.get("PREFETCH", "2"))
        nb_ = sum(caps) // BLOCK
        S0 = {}

        def dma_x0(i):
            xa = xp.tile([128, 3, BLOCK], BF16, tag="xa")
            nc.sync.dma_start(out=xa[:], in_=xT3[:, :, i * BLOCK : (i + 1) * BLOCK])
            return xa

        if WRAP:
            for i in range(min(PREFETCH, nb_)):
                S0[i] = {"xa": dma_x0(i)}
        loop_cm = tc.For_i(0, repeat, 1) if repeat > 1 else contextlib.nullcontext()
        with loop_cm:
            _emit_body(
                tc, xT3, w1s, w2s, b1s, b2m0s, b2m1s, ones, SD, with_bias, caps,
                xp, t1p, u1p, g1p, m2p, e2p, z1m0p, z1m1p, z2m0p, z2m1p,
                S0=S0,
            )
        nc.sync.dma_start(out=outS, in_=SD[:])


def _emit_body(
    tc, xT3, w1s, w2s, b1s, b2m0s, b2m1s, ones, SD, with_bias, caps,
    xp, t1p, u1p, g1p, m2p, e2p, z1m0p, z1m1p, z2m0p, z2m1p,
    S0=None,
):
    nc = tc.nc
    CB = COLS_PER_BLOCK
    ONLY = os.environ.get("ONLY", "full")
    nb = sum(caps) // BLOCK
    block_expert = []
    for e in range(E):
        block_expert += [e] * (caps[e] // BLOCK)

    u1eng = nc.gpsimd if U1ENG == "gpsimd" else nc.vector

    PREFETCH = int(os.environ.get("PREFETCH", "2"))
    WRAP = os.environ.get("WRAP", "0") == "1"
    S = dict(S0) if (WRAP and S0) else {}  # per-block pipeline state

    def dma_x(i):
        xa = xp.tile([128, 3, BLOCK], BF16, tag="xa")
        nc.sync.dma_start(
            out=xa[:], in_=xT3[:, :, i * BLOCK : (i + 1) * BLOCK]
        )
        return xa

    if not WRAP:
        for i in range(min(PREFETCH, nb)):
            S[i] = {"xa": dma_x(i)}

    STAGE_ORDER = os.environ.get("STAGE_ORDER", "p1_p2")

    def do_p1_m1(j):
        # t1/u1 must stay f32: u1 = t1-1 with t1 near 1.0 — bf16's
        # 2^-8 grid at 1.0 quantizes u1 (|u1| <= ~0.03 here) to a few
        # levels and the S1 sums blow past the error gate.
        st = S[j]
        z1m1 = st["z1m1"]
        t1 = t1p.tile([128, 1536], F32, tag="t1")
        u1 = u1p.tile([128, 1536], F32, tag="u1")
        g1k0 = g1p.tile([128, BLOCK], BF16, tag="g1k0")
        g1k1 = g1p.tile([128, HB], BF16, tag="g1k1")
        st["t1"], st["u1"] = t1, u1
        st["g1k0"], st["g1k1"] = g1k0, g1k1
        # m1 first: unblocks mm1m1(i) (single-buffered Z1M1) earliest
        if M1_SPLIT:
            # column halves: STT-m1-h0 completes while exp-h1 runs, so
            # z1m1 releases ~300ns earlier for MM1m1(i+1)
            for h in range(2):
                lo, hi = 1024 + QB * h, 1024 + QB * (h + 1)
                nc.scalar.activation(t1[:, lo:hi], z1m1[:, QB * h : QB * (h + 1)], AF.Exp)
                u1eng.tensor_scalar(
                    out=u1[:, lo:hi], in0=t1[:, lo:hi],
                    scalar1=-1.0, scalar2=0.0, op0=ALU.add, op1=ALU.min,
                )
                nc.vector.scalar_tensor_tensor(
                    out=g1k1[:, QB * h : QB * (h + 1)], in0=u1[:, lo:hi],
                    scalar=1.0, in1=z1m1[:, QB * h : QB * (h + 1)],
                    op0=ALU.mult, op1=ALU.max,
                    accum_out=SD[:, CB * j + 1 : CB * j + 2]
                    if h == 0 else SD[:, CB * j + 8 : CB * j + 9],
                )
        else:
            nc.scalar.activation(t1[:, 1024:1536], z1m1[:], AF.Exp)
            u1eng.tensor_scalar(
                out=u1[:, 1024:1536], in0=t1[:, 1024:1536],
                scalar1=-1.0, scalar2=0.0, op0=ALU.add, op1=ALU.min,
            )
            nc.vector.scalar_tensor_tensor(
                out=g1k1[:], in0=u1[:, 1024:1536], scalar=1.0, in1=z1m1[:],
                op0=ALU.mult, op1=ALU.max,
                accum_out=SD[:, CB * j + 1 : CB * j + 2],
            )

    def do_p1_m0(j):
        st = S[j]
        z1m0 = st["z1m0"]
        t1, u1, g1k0 = st["t1"], st["u1"], st["g1k0"]
        nc.scalar.activation(t1[:, 0:1024], z1m0[:], AF.Exp)
        if P1_SPLIT:
            # halve the m0 chain: g1k0-h0 releases early so MM2(i)'s
            # kt0-m0-h0 can start while h1 is still in pointwise.
            # accum columns per half are summed on the host (same dims).
            for h in range(2):
                u1eng.tensor_scalar(
                    out=u1[:, HB * h : HB * (h + 1)],
                    in0=t1[:, HB * h : HB * (h + 1)],
                    scalar1=-1.0, scalar2=0.0, op0=ALU.add, op1=ALU.min,
                )
                nc.vector.scalar_tensor_tensor(
                    out=g1k0[:, HB * h : HB * (h + 1)],
                    in0=u1[:, HB * h : HB * (h + 1)], scalar=1.0,
                    in1=z1m0[:, HB * h : HB * (h + 1)],
                    op0=ALU.mult, op1=ALU.max,
                    accum_out=SD[:, CB * j + 0 : CB * j + 1]
                    if h == 0 else SD[:, CB * j + 6 : CB * j + 7],
                )
        else:
            u1eng.tensor_scalar(
                out=u1[:, 0:1024], in0=t1[:, 0:1024],
                scalar1=-1.0, scalar2=0.0, op0=ALU.add, op1=ALU.min,
            )
            nc.vector.scalar_tensor_tensor(
                out=g1k0[:], in0=u1[:, 0:1024], scalar=1.0, in1=z1m0[:],
                op0=ALU.mult, op1=ALU.max,
                accum_out=SD[:, CB * j + 0 : CB * j + 1],
            )

    DMA_FIRST = os.environ.get("DMA_FIRST", "0") == "1"
    for i in range(nb + 2):
        j = i - 1
        k = i - 2
        if DMA_FIRST and i + PREFETCH < nb:
            S[i + PREFETCH] = {"xa": dma_x(i + PREFETCH)}
        p1_ok = 0 <= j < nb and ONLY in ("p1", "mm2", "full")
        # ---- stage P1(i-1) m1 chain (latency-critical) ----
        if p1_ok:
            do_p1_m1(j)
        if STAGE_ORDER == "p1_p2" and p1_ok:
            do_p1_m0(j)

        # ---- stage P2(i-2): layer-2 min/exp + fused column sums ----
        if 0 <= k < nb and ONLY == "full":
            st = S.pop(k)
            e = block_expert[k]
            z2m0, z2m1 = st["z2m0"], st["z2m1"]
            m2 = m2p.tile([128, 1280], F32, tag="m2")
            e2 = e2p.tile([128, 1280], F32, tag="e2")
            if with_bias:
                # m1 A-form: m2 = min(z2 + b2, 0); b2 per-partition scalar AP
                nc.vector.tensor_scalar(
                    out=m2[:, 1024:1280], in0=z2m1[:],
                    scalar1=b2m1s[e][:], scalar2=0.0, op0=ALU.add, op1=ALU.min,
                    accum_out=SD[:, CB * k + 3 : CB * k + 4],
                )
                nc.scalar.activation(
                    e2[:, 1024:1280], m2[:, 1024:1280], AF.Exp,
                    accum_out=SD[:, CB * k + 5 : CB * k + 6],
                )
                nc.vector.tensor_scalar(
                    out=m2[:, 0:1024], in0=z2m0[:],
                    scalar1=b2m0s[e][:], scalar2=0.0, op0=ALU.add, op1=ALU.min,
                    accum_out=SD[:, CB * k + 2 : CB * k + 3],
                )
            else:
                # m1 B-form, pure ACT: r2 = relu(-z2) (accum = -Smin_m1),
                # then exp(-r2) = exp(min(z2,0)) (accum = Sexp_m1).
                nc.scalar.activation(
                    m2[:, 1024:1280], z2m1[:], AF.Relu, scale=-1.0,
                    accum_out=SD[:, CB * k + 3 : CB * k + 4],
                )
                nc.scalar.activation(
                    e2[:, 1024:1280], m2[:, 1024:1280], AF.Exp, scale=-1.0,
                    accum_out=SD[:, CB * k + 5 : CB * k + 6],
                )
                if P2_SPLIT and P1_SPLIT:
                    # halves: TS-h0 starts after kt1-h0 stop; both accum
                    # columns carry the same dims (summed with col 2/6 pairs
                    # on the host via the h1 spare column)
                    nc.vector.tensor_scalar(
                        out=m2[:, 0:512], in0=z2m0[:, 0:512],
                        scalar1=0.0, scalar2=0.0, op0=ALU.min, op1=ALU.add,
                        accum_out=SD[:, CB * k + 2 : CB * k + 3],
                    )
                    nc.vector.tensor_scalar(
                        out=m2[:, 512:1024], in0=z2m0[:, 512:1024],
                        scalar1=0.0, scalar2=0.0, op0=ALU.min, op1=ALU.add,
                        accum_out=SD[:, CB * k + 7 : CB * k + 8],
                    )
                else:
                    nc.vector.tensor_scalar(
                        out=m2[:, 0:1024], in0=z2m0[:],
                        scalar1=0.0, scalar2=0.0, op0=ALU.min, op1=ALU.add,
                        accum_out=SD[:, CB * k + 2 : CB * k + 3],
                    )
            nc.scalar.activation(
                e2[:, 0:1024], m2[:, 0:1024], AF.Exp,
                accum_out=SD[:, CB * k + 4 : CB * k + 5],
            )

        # ---- stage P1(i-1) m0 chain, after P2 when reordered ----
        if STAGE_ORDER != "p1_p2" and p1_ok:
            do_p1_m0(j)

        # ---- stage MM1(i) ----
        if i < nb and ONLY != "dma":
            st = S[i]
            e = block_expert[i]
            w1 = w1s[e]
            xa = st["xa"]
            z1m0 = z1m0p.tile([128, BLOCK], F32, tag="z1m0")
            z1m1 = z1m1p.tile([128, HB], F32, tag="z1m1")
            last = 2 if not with_bias else -1
            MM1_ORDER = os.environ.get("MM1_ORDER", "m1_first")
            if MM1_ORDER == "m1_first":
                for kt in range(3):
                    for h in range(2):
                        nc.tensor.matmul(
                            z1m1[64 * h : 64 * (h + 1), :],
                            lhsT=w1[:, kt, 128:192],
                            rhs=xa[:, kt, HB * h : HB * (h + 1)],
                            start=(kt == 0), stop=(kt == last),
                        )
                for kt in range(3):
                    for h in range(2):
                        nc.tensor.matmul(
                            z1m0[:, HB * h : HB * (h + 1)],
                            lhsT=w1[:, kt, 0:128],
                            rhs=xa[:, kt, HB * h : HB * (h + 1)],
                            start=(kt == 0), stop=(kt == last),
                        )
            elif MM1_ORDER == "interleave":
                for kt in range(3):
                    for h in range(2):
                        nc.tensor.matmul(
                            z1m0[:, HB * h : HB * (h + 1)],
                            lhsT=w1[:, kt, 0:128],
                            rhs=xa[:, kt, HB * h : HB * (h + 1)],
                            start=(kt == 0), stop=(kt == last),
                        )
                    for h in range(2):
                        nc.tensor.matmul(
                            z1m1[64 * h : 64 * (h + 1), :],
                            lhsT=w1[:, kt, 128:192],
                            rhs=xa[:, kt, HB * h : HB * (h + 1)],
                            start=(kt == 0), stop=(kt == last),
                        )
            else:  # m0 first, m1 grouped at the end
                for kt in range(3):
                    for h in range(2):
                        nc.tensor.matmul(
                            z1m0[:, HB * h : HB * (h + 1)],
                            lhsT=w1[:, kt, 0:128],
                            rhs=xa[:, kt, HB * h : HB * (h + 1)],
                            start=(kt == 0), stop=(kt == last),
                        )
                for kt in range(3):
                    for h in range(2):
                        nc.tensor.matmul(
                            z1m1[64 * h : 64 * (h + 1), :],
                            lhsT=w1[:, kt, 128:192],
                            rhs=xa[:, kt, HB * h : HB * (h + 1)],
                            start=(kt == 0), stop=(kt == last),
                        )
            if with_bias:
                b1 = b1s[e]
                for h in range(2):
                    nc.tensor.matmul(
                        z1m0[:, HB * h : HB * (h + 1)], lhsT=b1[:, 0:128],
                        rhs=ones[:], start=False, stop=True,
                    )
                for h in range(2):
                    nc.tensor.matmul(
                        z1m1[64 * h : 64 * (h + 1), :], lhsT=b1[:, 128:192],
                        rhs=ones[:], start=False, stop=True,
                    )
            st["z1m0"], st["z1m1"] = z1m0, z1m1

        # ---- stage MM2(i-1) ----
        if 0 <= j < nb and ONLY in ("mm2", "full"):
            st = S[j]
            e = block_expert[j]
            w2 = w2s[e]
            g1k0, g1k1 = st["g1k0"], st["g1k1"]
            z2m0 = z2m0p.tile([128, BLOCK], F32, tag="z2m0")
            # m1 packed as four 256-atom quarters: dim 128+d at
            # partition 32q+d, atom = 256q + col
            z2m1 = z2m1p.tile([128, QB], F32, tag="z2m1")
            if os.environ.get("MM2_ORDER", "kt1_first") == "kt1_first":
                # kt1 depends only on g1k1 (its STT completes before
                # g1k0's in P1), so its matmuls are ready first.
                for h in range(2):
                    nc.tensor.matmul(
                        z2m0[:, HB * h : HB * (h + 1)],
                        lhsT=w2[64 * h : 64 * h + 64, 1, 0:128],
                        rhs=g1k1[64 * h : 64 * (h + 1), :],
                        start=True, stop=False,
                    )
                for q in range(4):
                    h, r = q >> 1, q & 1
                    nc.tensor.matmul(
                        z2m1[32 * q : 32 * q + 32, :],
                        lhsT=w2[64 * h : 64 * h + 64, 1, 128:160],
                        rhs=g1k1[64 * h : 64 * (h + 1), QB * r : QB * (r + 1)],
                        start=True, stop=False,
                        tile_position=(64 * h, 32 * q),
                    )
                for h in range(2):
                    nc.tensor.matmul(
                        z2m0[:, HB * h : HB * (h + 1)], lhsT=w2[:, 0, 0:128],
                        rhs=g1k0[:, HB * h : HB * (h + 1)],
                        start=False, stop=True,
                    )
                for q in range(4):
                    nc.tensor.matmul(
                        z2m1[32 * q : 32 * q + 32, :],
                        lhsT=w2[:, 0, 128:160],
                        rhs=g1k0[:, QB * q : QB * (q + 1)],
                        start=False, stop=True,
                        tile_position=(0, 32 * q),
                    )
            else:
                # kt0 m0 (LDW w2[:,0,0:128])
                for h in range(2):
                    nc.tensor.matmul(
                        z2m0[:, HB * h : HB * (h + 1)], lhsT=w2[:, 0, 0:128],
                        rhs=g1k0[:, HB * h : HB * (h + 1)],
                        start=True, stop=False,
                    )
                # kt0 m1: quarter q at PE col-tile (0, 32q)
                for q in range(4):
                    nc.tensor.matmul(
                        z2m1[32 * q : 32 * q + 32, :],
                        lhsT=w2[:, 0, 128:160],
                        rhs=g1k0[:, QB * q : QB * (q + 1)],
                        start=True, stop=False,
                        tile_position=(0, 32 * q),
                    )
                # kt1 m0; g1k1 parts: h half of atoms, lhsT at matching base
                if os.environ.get("KT1Q", "0") == "1":
                    for h in range(2):
                        for c in range(2):
                            nc.tensor.matmul(
                                z2m0[:, HB * h + QB * c : HB * h + QB * (c + 1)],
                                lhsT=w2[64 * h : 64 * h + 64, 1, 0:128],
                                rhs=g1k1[64 * h : 64 * (h + 1), QB * c : QB * (c + 1)],
                                start=False, stop=True,
                            )
                else:
                    for h in range(2):
                        nc.tensor.matmul(
                            z2m0[:, HB * h : HB * (h + 1)],
                            lhsT=w2[64 * h : 64 * h + 64, 1, 0:128],
                            rhs=g1k1[64 * h : 64 * (h + 1), :],
                            start=False, stop=True,
                        )
                # kt1 m1: quarter q = (h, r): rhs g1k1[64h:64h+64, 256r:..],
                # PE tile (64h, 32q) — row+col compose
                for q in range(4):
                    h, r = q >> 1, q & 1
                    nc.tensor.matmul(
                        z2m1[32 * q : 32 * q + 32, :],
                        lhsT=w2[64 * h : 64 * h + 64, 1, 128:160],
                        rhs=g1k1[64 * h : 64 * (h + 1), QB * r : QB * (r + 1)],
                        start=False, stop=True,
                        tile_position=(64 * h, 32 * q),
                    )
            st["z2m0"], st["z2m1"] = z2m0, z2m1

        # ---- prefetch ----
        if WRAP:
            # wraparound: the last PREFETCH iterations prefetch blocks
            # 0..PREFETCH-1 for the NEXT repeat iteration.  Requires
            # X_BUFS dividing (nb + PREFETCH) so the wraparound DMA lands
            # on the same SBUF slot the fixed block-0 consumer reads.
            if i < nb:
                S[(i + PREFETCH) % nb] = {"xa": dma_x((i + PREFETCH) % nb)}
        elif not DMA_FIRST:
            nxt = i + PREFETCH
            if nxt < nb:
                S[nxt] = {"xa": dma_x(nxt)}


_GRAPH_CACHE = {}


def _get_graph(with_bias: bool, caps):
    key = (with_bias, tuple(caps))
    if key not in _GRAPH_CACHE:
        _GRAPH_CACHE[key] = _build_graph(with_bias, caps)
    return _GRAPH_CACHE[key]


def _celu64(v):
    return np.where(v > 0, v, np.expm1(np.minimum(v, 0.0)))


def prepare_in_maps(aev_inputs, atom_types, W1, b1, W2, b2, W3, b3):
    """Host routing: build per-core input maps + metadata for decode."""
    import ml_dtypes

    ndt = ml_dtypes.bfloat16
    aev = np.asarray(aev_inputs, dtype=np.float32)
    types = np.asarray(atom_types).astype(np.int64)
    W1f = np.asarray(W1, dtype=np.float32)
    b1 = np.asarray(b1, dtype=np.float32)
    W2f = np.asarray(W2, dtype=np.float32)
    b2 = np.asarray(b2, dtype=np.float32)
    W3f = np.asarray(W3, dtype=np.float32)
    b3 = np.asarray(b3, dtype=np.float32)
    W1b = np.ascontiguousarray(W1f.astype(ndt))
    W2b = np.ascontiguousarray(W2f.astype(ndt))

    with_bias = bool(np.any(b1) or np.any(b2))

    order = np.argsort(types, kind="stable")
    sorted_types = types[order]
    bounds = np.searchsorted(sorted_types, np.arange(E + 1))
    type_lists = [order[bounds[e] : bounds[e + 1]] for e in range(E)]

    SHED_MAX = 192
    slices = [[None] * E for _ in range(N_CORES)]
    n_real = np.zeros((N_CORES, E), dtype=np.int64)
    shed = []
    caps = []
    for e in range(E):
        lst = type_lists[e]
        counts = [
            ((len(lst) * (c + 1)) // N_CORES) - ((len(lst) * c) // N_CORES)
            for c in range(N_CORES)
        ]
        mx = max(counts)
        rem = mx % BLOCK
        if 0 < rem <= SHED_MAX:
            cap_e = (mx // BLOCK) * BLOCK
        else:
            cap_e = -(-mx // BLOCK) * BLOCK
        caps.append(cap_e)
        for c in range(N_CORES):
            lo = (len(lst) * c) // N_CORES
            hi = (len(lst) * (c + 1)) // N_CORES
            take = min(hi - lo, cap_e)
            slices[c][e] = lst[lo : lo + take]
            shed.append(lst[lo + take : hi])
            n_real[c, e] = take
    shed = np.concatenate(shed) if shed else np.zeros(0, dtype=np.int64)
    caps = tuple(caps)
    offs = np.cumsum([0] + list(caps))

    shed_energy = 0.0
    if len(shed):
        xs = aev[shed].astype(np.float64)
        ts_ = types[shed]
        for e in range(E):
            m = ts_ == e
            if not m.any():
                continue
            h = _celu64(xs[m] @ W1f[e].astype(np.float64) + b1[e].astype(np.float64))
            h = _celu64(h @ W2f[e].astype(np.float64) + b2[e].astype(np.float64))
            y = h @ W3f[e].astype(np.float64)[:, 0] + float(b3[e][0])
            shed_energy += float(y.sum())

    in_maps = []
    for c in range(N_CORES):
        xcT = np.zeros((IN_DIM, int(offs[-1])), dtype=ndt)
        for e in range(E):
            idx = slices[c][e]
            xcT[:, int(offs[e]) : int(offs[e]) + len(idx)] = aev[idx].T.astype(ndt)
        m = {"xT": xcT, "W1": W1b, "W2": W2b}
        if with_bias:
            m["B1"] = np.ascontiguousarray(b1.astype(ndt))
            m["B2"] = np.ascontiguousarray(b2.astype(np.float32))
        in_maps.append(m)
    return in_maps, n_real, with_bias, (b1, W2f, b2, W3f, b3, shed_energy), caps


def postprocess(results, n_real, wdata, caps, with_bias=False):
    """Decode per-block accum columns -> per-expert energies (f64).

    Per block k, SD columns 6k..6k+5 hold:
      0: S1_m0   [128]  sum over atoms of g1 (dims 0:128)
      1: S1_m1   [128]  folded: dim 128+j = col[j] + col[64+j]
      2: Smin_m0 [128]  sum of min(z2+b2, 0), dims 0:128
      3: Smin_m1 [128]  quarters: dim 128+j = sum_q col[32q+j]
         (zero-bias B-form: column holds sum relu(-z2) = -Smin_m1)
      4: Sexp_m0 [128]  sum of exp(min(z2+b2, 0))
      5: Sexp_m1 [128]  quarters, as 3
    """
    b1, W2f, b2, W3f, b3, shed_energy = wdata
    CB = COLS_PER_BLOCK
    nb = sum(caps) // BLOCK
    block_expert = []
    for e in range(E):
        block_expert += [e] * (caps[e] // BLOCK)

    S1 = np.zeros((E, H1), dtype=np.float64)
    Smin = np.zeros((E, H2), dtype=np.float64)
    Sexp = np.zeros((E, H2), dtype=np.float64)
    for c in range(N_CORES):
        D = np.asarray(results[c]["outS"], dtype=np.float64)
        for k in range(nb):
            e = block_expert[k]
            S1[e, 0:128] += D[:, CB * k]
            if P1_SPLIT:
                S1[e, 0:128] += D[:, CB * k + 6]
            if P2_SPLIT:
                Smin[e, 0:128] += D[:, CB * k + 7]
            S1[e, 128:192] += D[0:64, CB * k + 1] + D[64:128, CB * k + 1]
            if M1_SPLIT:
                S1[e, 128:192] += D[0:64, CB * k + 8] + D[64:128, CB * k + 8]
            Smin[e, 0:128] += D[:, CB * k + 2]
            m1col = D[:, CB * k + 3].reshape(4, 32).sum(axis=0)
            Smin[e, 128:160] += m1col if with_bias else -m1col
            Sexp[e, 0:128] += D[:, CB * k + 4]
            Sexp[e, 128:160] += D[:, CB * k + 5].reshape(4, 32).sum(axis=0)

    total = shed_energy
    counts_e = n_real.sum(axis=0)
    for e in range(E):
        ncols = float(N_CORES * caps[e])  # real + pad columns on device
        pads = ncols - float(counts_e[e])
        w3 = W3f[e].astype(np.float64)[:, 0]
        b2e = b2[e].astype(np.float64)
        # sum z2 over all device columns: W2^T S1 + ncols*b2
        sz2 = W2f[e].astype(np.float64).T @ S1[e] + ncols * b2e
        S = (sz2 - Smin[e]) + Sexp[e] - ncols  # sum of celu(z2+b2) per dim
        total += float(w3 @ S)
        total += float(counts_e[e]) * float(b3[e][0])
        if pads:
            # device pads contribute celu(z2_0) per dim; subtract (f64 model)
            h1 = _celu64(b1[e].astype(np.float64))
            z2_0 = h1 @ W2f[e].astype(np.float64) + b2e
            total -= pads * float(w3 @ _celu64(z2_0))
    return np.asarray(total, dtype=np.float32)


def kernel(aev_inputs, atom_types, W1, b1, W2, b2, W3, b3):
    in_maps, n_real, with_bias, wdata, caps = prepare_in_maps(
        aev_inputs, atom_types, W1, b1, W2, b2, W3, b3
    )
    nc = _get_graph(with_bias, caps)
    results = bass_utils.run_bass_kernel_spmd(
        nc, in_maps, core_ids=list(range(N_CORES))
    ).results
    return postprocess(results, n_real, wdata, caps, with_bias)
